# revision 2
# baseline (speedup 1.0000x reference)
"""Gammatone filterbank as truncated-FIR matmuls on the PE — v3.

Same math as kernel2 (512-tap bf16 FIR, diagonal shifted-x tiles, 4
accumulating matmuls per 512-col PSUM tile), but:
  - x arrives host-side zero-padded and pre-cast to bf16 ("xq"), so the
    device has no staging prologue at all;
  - chunk sizes ramp 512/1024/2048 -> 4096 so the PE starts ~3us in;
  - S loads and output writes rotate across the three DMA-capable queues
    (SP / Activation / Pool); PSUM->SBUF copies run on the DVE.
"""

import sys

import numpy as np

for _p in ("/opt/trn_rl_repo",):
    if _p not in sys.path:
        sys.path.insert(0, _p)

import ml_dtypes
import concourse.bass as bass
import concourse.mybir as mybir
from concourse.bacc import Bacc
from concourse.bass_utils import run_bass_kernel_spmd
from concourse.tile import TileContext

B = 8
T = 32000
C = 128
NTAP = 512
NB = NTAP // 128
PAD = 512
NCOL = 512
F32 = mybir.dt.float32
BF16 = mybir.dt.bfloat16


def chunk_sizes(t_len):
    sizes = [512, 1024, 2048]
    done = sum(sizes)
    while t_len - done > 4096:
        sizes.append(4096)
        done += 4096
    if t_len - done:
        sizes.append(t_len - done)
    return sizes


def build_bass(t_len=T):
    nc = Bacc()
    xq = nc.declare_dram_parameter("xq", [1, PAD + t_len], BF16, isOutput=False)
    wts = nc.declare_dram_parameter("wts", [128, NTAP], BF16, isOutput=False)
    out = nc.declare_dram_parameter("out", [C, t_len], F32, isOutput=True)

    queues = ("sync", "scalar", "gpsimd")

    with TileContext(nc) as tc:
        with (
            tc.tile_pool(name="consts", bufs=1) as consts,
            tc.tile_pool(name="sload", bufs=3) as sload,
            tc.tile_pool(name="osb", bufs=4) as osb,
            tc.tile_pool(name="psum", bufs=4, space="PSUM") as psp,
        ):
            w = consts.tile([128, NTAP], BF16, tag="w", name="w")
            nc.scalar.dma_start(out=w[:], in_=wts[:])

            tile_idx = 0
            t0 = 0
            for p, ncp in enumerate(chunk_sizes(t_len)):
                sc = ncp + NTAP - 128
                s = sload.tile([128, sc], BF16, tag="S", name=f"s{p}")
                ssrc = bass.AP(tensor=xq, offset=PAD + t0 - (NTAP - 1),
                               ap=[[1, 128], [1, sc]])
                getattr(nc, queues[p % 3]).dma_start(out=s[:], in_=ssrc)

                for j0 in range(0, ncp, NCOL):
                    wc = min(NCOL, ncp - j0)
                    acc = psp.tile([128, NCOL], F32, tag="acc", name="acc")
                    for b in range(NB):
                        u0 = (NTAP - 128) - 128 * b + j0
                        nc.tensor.matmul(
                            acc[:, 0:wc],
                            w[:, 128 * b:128 * b + 128],
                            s[:, u0:u0 + wc],
                            start=(b == 0), stop=(b == NB - 1))
                    res = osb.tile([128, NCOL], F32, tag="res", name="res")
                    nc.vector.tensor_copy(res[:, 0:wc], acc[:, 0:wc])
                    qe = queues[(tile_idx + 1) % 3]
                    getattr(nc, qe).dma_start(
                        out=out[:, t0 + j0:t0 + j0 + wc], in_=res[:, 0:wc])
                    tile_idx += 1
                t0 += ncp
    nc.finalize()
    return nc


def make_weights(coef_re, coef_im, factor):
    """wts[p, 128b + c] = g[c, 128b + 127 - p]; g = truncated gammatone IR."""
    cr = np.asarray(coef_re, np.float64)
    ci = np.asarray(coef_im, np.float64)
    f = np.asarray(factor, np.float64)
    c = cr + 1j * ci
    n = np.arange(NTAP)
    binom = (n + 3) * (n + 2) * (n + 1) / 6.0
    g = f[:, None] * np.real(binom[None, :] * c[:, None] ** n[None, :])
    g = g.astype(np.float32)
    wv = np.zeros((128, NTAP), np.float32)
    for b in range(NB):
        blk = g[:, 128 * b:128 * b + 128]
        wv[:, 128 * b:128 * b + 128] = blk[:, ::-1].T
    return wv.astype(ml_dtypes.bfloat16)


_CACHED_NC = None


def kernel(inp, coef_re, coef_im, factor):
    global _CACHED_NC
    inp = np.ascontiguousarray(np.asarray(inp, np.float32))
    assert inp.shape == (B, T)
    wv = make_weights(coef_re, coef_im, factor)
    xq = np.zeros((B, PAD + T), np.float32)
    xq[:, PAD:] = inp
    xq16 = xq.astype(ml_dtypes.bfloat16)

    if _CACHED_NC is None:
        _CACHED_NC = build_bass()
    nc = _CACHED_NC

    in_maps = [
        {"xq": xq16[i:i + 1, :], "wts": wv}
        for i in range(B)
    ]
    res = run_bass_kernel_spmd(nc, in_maps, core_ids=list(range(B)))
    out = np.stack([np.asarray(res.results[i]["out"]).T for i in range(B)])
    return np.ascontiguousarray(out.astype(np.float32))


# revision 3
# speedup vs baseline: 1.4876x; 1.4876x over previous
"""Gammatone filterbank (4 cascaded complex one-pole IIR sections) on TRN2.

Two cooperating compute paths per core (one waveform, all 128 bands):

FIR head (columns [0, T1)):  the 4th-order gammatone IIR per band c has
impulse response h[c, n] = factor_c * Re[C(n+3,3) coef_c^n], which decays
geometrically (|coef| <= 0.984).  Truncated at NTAP=512 taps (max rel err
~9e-3, gate 2e-2) the filterbank is a dense FIR evaluated on the PE as
matmuls: tap index n = 128*b + r contracts over the partition axis using a
"diagonal" SBUF tile S[p, u] = x(t0 - 511 + u + p) loaded with a single
overlapping-window DMA; all 4 tap blocks read S at different column
offsets against stationary bf16 weights W_b[p, c] = g[c, 128b + 127 - p].
4 accumulating bf16 matmuls per 512-column PSUM tile = 1 PE cycle/column.

IIR-scan tail (columns [T1, T)):  computed exactly on the otherwise-idle
DVE/Pool engines with the derotated-scan trick: with sh[j] = s[j]e^{-ij b}
the complex recurrence becomes real scans with coefficient lam = |coef|,
so the 4-stage cascade is 8 first-order tensor_tensor_scans (re-chain on
DVE, im-chain on Pool) plus modulation/demodulation table multiplies.
The section starts from zero state W=640 columns early (warmup converges
the IIR state to ~2e-3 of scale before the first kept column); pieces of
KP=512 columns carry state via an e^{+i KP b} rotation ([C,8] ops) so the
mod/demod tables are piece-local and loaded once.  Scan-piece ops are
emitted interleaved with the FIR tiles so the in-order engine queues
overlap everything.

Inputs are staged host-side (zero-padded bf16 x for the diagonal loads,
f32 x slice for the scan section, tap/table constants) - pure data prep,
all filtering runs on device.  DMA queues: SP/Act carry S loads and output
writes, Pool carries constants/broadcasts; PSUM->SBUF copies split
DVE/Pool 1:2.

Sharding: batch-parallel SPMD, one waveform per NeuronCore (8 cores, B=8).
Output is [C, T] per core; the host transposes/stacks to [B, T, C].
"""

import sys

import numpy as np

for _p in ("/opt/trn_rl_repo",):
    if _p not in sys.path:
        sys.path.insert(0, _p)

import ml_dtypes
import concourse.bass as bass
import concourse.mybir as mybir
from concourse.bacc import Bacc
from concourse.bass_utils import run_bass_kernel_spmd
from concourse.tile import TileContext

B = 8
T = 32000
C = 128
NTAP = 512            # FIR taps (4 blocks of 128)
NB = NTAP // 128
PAD = 512             # leading zeros in xq
NCOL = 512            # PSUM tile columns (one bank)
KP = 512              # scan piece length
NPIECE = 8
W = 640               # scan warmup columns (discarded)
KSEC = KP * NPIECE
T2 = KSEC - W
T1 = T - T2
COPY_MOD = 3          # 1/3 of PSUM copies on DVE, rest on Pool
PSUM_BUFS = 6
START_TILE = 2        # first FIR tile index that emits scan steps
END_MARGIN = 2        # scan steps finish this many tiles early
F32 = mybir.dt.float32
BF16 = mybir.dt.bfloat16
MULT = mybir.AluOpType.mult
ADD = mybir.AluOpType.add
SUB = mybir.AluOpType.subtract


def chunk_sizes(t_len):
    """FIR S-chunk sizes: ramp up for fast PE start, 256 tail for drain."""
    sizes = [256, 512, 1024, 2048]
    done = sum(sizes)
    while t_len - done > 4096 + 256:
        sizes.append(4096)
        done += 4096
    rest = t_len - done - 256
    if rest > 0:
        sizes.append(rest)
    sizes.append(256)
    assert sum(sizes) == t_len, (sizes, t_len)
    return sizes


def build_bass():
    nc = Bacc()
    xq = nc.declare_dram_parameter("xq", [1, PAD + T], BF16, isOutput=False)
    wts = nc.declare_dram_parameter("wts", [128, NTAP], BF16, isOutput=False)
    xr32 = nc.declare_dram_parameter("xr32", [1, KSEC], F32, isOutput=False)
    mclp = nc.declare_dram_parameter("mcl", [C, KP], F32, isOutput=False)
    mslp = nc.declare_dram_parameter("msl", [C, KP], F32, isOutput=False)
    lamp = nc.declare_dram_parameter("lamt", [C, KP], F32, isOutput=False)
    ckp = nc.declare_dram_parameter("ck", [C, 1], F32, isOutput=False)
    skp = nc.declare_dram_parameter("sk", [C, 1], F32, isOutput=False)
    out = nc.declare_dram_parameter("out", [C, T], F32, isOutput=True)

    with TileContext(nc) as tc:
        with (
            tc.tile_pool(name="consts", bufs=1) as consts,
            tc.tile_pool(name="sload", bufs=3) as sload,
            tc.tile_pool(name="osb", bufs=6) as osb,
            tc.tile_pool(name="xbp", bufs=2) as xbp,
            tc.tile_pool(name="modp", bufs=2) as modp,
            tc.tile_pool(name="workr", bufs=2) as workr,
            tc.tile_pool(name="worki", bufs=2) as worki,
            tc.tile_pool(name="zp", bufs=2) as zp,
            tc.tile_pool(name="states", bufs=2) as stp,
            tc.tile_pool(name="psum", bufs=PSUM_BUFS, space="PSUM") as psp,
        ):
            # FIR weights on the Act queue (needed first); scan-section
            # constants on the Pool queue, which is idle early.
            w = consts.tile([128, NTAP], BF16, tag="w", name="w")
            nc.scalar.dma_start(out=w[:], in_=wts[:])
            tabc = consts.tile([C, KP], F32, tag="mcl", name="tabc")
            tabs = consts.tile([C, KP], F32, tag="msl", name="tabs")
            lam_t = consts.tile([C, KP], F32, tag="lam", name="lam_t")
            ck = consts.tile([C, 1], F32, tag="ck", name="ck")
            sk = consts.tile([C, 1], F32, tag="sk", name="sk")
            nc.gpsimd.dma_start(out=tabc[:], in_=mclp[:])
            nc.gpsimd.dma_start(out=tabs[:], in_=mslp[:])
            nc.gpsimd.dma_start(out=lam_t[:], in_=lamp[:])
            nc.gpsimd.dma_start(out=ck[:], in_=ckp[:])
            nc.gpsimd.dma_start(out=sk[:], in_=skp[:])
            st_rot0 = stp.tile([C, 8], F32, tag="st_rot", name="st0")
            nc.vector.memset(st_rot0[:], 0.0)

            # ---- scan-section step closures (emitted interleaved) ----
            steps = []
            state = {"st_rot": st_rot0}
            for p in range(NPIECE):
                last = p == NPIECE - 1
                holder = {}

                def s_xb(p=p, holder=holder):
                    xb = xbp.tile([C, KP], F32, tag="xb", name=f"xb{p}")
                    src = bass.AP(tensor=xr32, offset=p * KP,
                                  ap=[[0, C], [1, KP]])
                    nc.gpsimd.dma_start(out=xb[:], in_=src)
                    holder["xb"] = xb

                def s_mr(p=p, holder=holder):
                    mr = modp.tile([C, KP], F32, tag="mr", name=f"mr{p}")
                    nc.gpsimd.tensor_tensor(mr[:], tabc[:], holder["xb"][:],
                                            MULT)
                    holder["r"] = mr

                def s_mi(p=p, holder=holder):
                    mi = modp.tile([C, KP], F32, tag="mi", name=f"mi{p}")
                    nc.vector.tensor_tensor(mi[:], tabs[:], holder["xb"][:],
                                            MULT)
                    holder["i"] = mi

                def s_straw(p=p, holder=holder):
                    holder["st_raw"] = stp.tile([C, 8], F32, tag="st_raw",
                                                name=f"sr{p}")

                def mk_stage(stage, p=p, last=last, holder=holder):
                    def s_r():
                        nr = workr.tile([C, KP], F32, tag=f"nr{stage % 2}",
                                        name=f"nr{p}_{stage}")
                        nc.vector.tensor_tensor_scan(
                            nr[:], lam_t[:], holder["r"][:],
                            state["st_rot"][:, stage:stage + 1], MULT, ADD)
                        if not last:
                            nc.vector.tensor_copy(
                                holder["st_raw"][:, stage:stage + 1],
                                nr[:, KP - 1:KP])
                        holder["r"] = nr

                    def s_i():
                        ni = worki.tile([C, KP], F32, tag=f"ni{stage % 2}",
                                        name=f"ni{p}_{stage}")
                        nc.gpsimd.tensor_tensor_scan(
                            ni[:], lam_t[:], holder["i"][:],
                            state["st_rot"][:, 4 + stage:5 + stage],
                            MULT, ADD)
                        if not last:
                            nc.gpsimd.tensor_copy(
                                holder["st_raw"][:, 4 + stage:5 + stage],
                                ni[:, KP - 1:KP])
                        holder["i"] = ni
                    return s_r, s_i

                def s_rot(p=p, holder=holder):
                    # carried state rotated by e^{+i*KP*beta}:
                    # re' = re*cK - im*sK ; im' = im*cK + re*sK
                    tmp = stp.tile([C, 8], F32, tag="st_tmp", name=f"tm{p}")
                    nxt = stp.tile([C, 8], F32, tag="st_rot", name=f"nx{p}")
                    sr = holder["st_raw"]
                    nc.vector.tensor_scalar(tmp[:, 0:4], sr[:, 4:8], sk[:],
                                            None, MULT)
                    nc.vector.tensor_scalar(tmp[:, 4:8], sr[:, 0:4], sk[:],
                                            None, MULT)
                    nc.vector.scalar_tensor_tensor(
                        nxt[:, 0:4], sr[:, 0:4], ck[:], tmp[:, 0:4],
                        MULT, SUB)
                    nc.vector.scalar_tensor_tensor(
                        nxt[:, 4:8], sr[:, 4:8], ck[:], tmp[:, 4:8],
                        MULT, ADD)
                    state["st_rot"] = nxt

                def s_zr(p=p, holder=holder):
                    zr = zp.tile([C, KP], F32, tag="zr", name=f"zr{p}")
                    nc.gpsimd.tensor_tensor(zr[:], tabc[:], holder["r"][:],
                                            MULT)
                    holder["zr"] = zr

                def s_zi(p=p, holder=holder):
                    zi = zp.tile([C, KP], F32, tag="zi", name=f"zi{p}")
                    nc.vector.tensor_tensor(zi[:], tabs[:], holder["i"][:],
                                            MULT)
                    holder["zi"] = zi

                def s_z(p=p, holder=holder):
                    z = zp.tile([C, KP], F32, tag="z", name=f"z{p}")
                    nc.gpsimd.tensor_tensor(z[:], holder["zr"][:],
                                            holder["zi"][:], ADD)
                    glo = p * KP
                    z_lo = max(0, min(KP, W - glo))
                    if z_lo < KP:
                        ot0 = T1 + glo + z_lo - W
                        nc.sync.dma_start(out=out[:, ot0:ot0 + KP - z_lo],
                                          in_=z[:, z_lo:KP])

                ss = [(s_xb,), (s_straw,), (s_mr,), (s_mi,)]
                for stage in range(4):
                    r_, i_ = mk_stage(stage)
                    ss += [(i_,), (r_,)]
                if not last:
                    ss.append((s_rot,))
                ss += [(s_zr,), (s_zi,), (s_z,)]
                steps += ss

            # ---- FIR emission with interleaved scan steps ----
            queues = ("sync", "scalar")
            n_tiles_total = sum((s + NCOL - 1) // NCOL
                                for s in chunk_sizes(T1))
            ti = 0
            t0 = 0
            si = 0
            for p, ncp in enumerate(chunk_sizes(T1)):
                sc = ncp + NTAP - 128
                s = sload.tile([128, sc], BF16, tag="S", name=f"s{p}")
                ssrc = bass.AP(tensor=xq, offset=PAD + t0 - (NTAP - 1),
                               ap=[[1, 128], [1, sc]])
                getattr(nc, queues[p % 2]).dma_start(out=s[:], in_=ssrc)
                for j0 in range(0, ncp, NCOL):
                    wc = min(NCOL, ncp - j0)
                    acc = psp.tile([128, NCOL], F32, tag="acc", name="acc")
                    for b in range(NB):
                        u0 = (NTAP - 128) - 128 * b + j0
                        nc.tensor.matmul(acc[:, 0:wc],
                                         w[:, 128 * b:128 * b + 128],
                                         s[:, u0:u0 + wc],
                                         start=(b == 0), stop=(b == NB - 1))
                    res = osb.tile([128, NCOL], F32, tag="res", name="res")
                    ceng = nc.vector if ti % COPY_MOD == 0 else nc.gpsimd
                    ceng.tensor_copy(res[:, 0:wc], acc[:, 0:wc])
                    getattr(nc, queues[(ti + 1) % 2]).dma_start(
                        out=out[:, t0 + j0:t0 + j0 + wc], in_=res[:, 0:wc])
                    ti += 1
                    if ti >= START_TILE and si < len(steps):
                        avail = max(1, n_tiles_total - START_TILE - END_MARGIN)
                        target = min((ti - START_TILE + 1) * len(steps)
                                     // avail, len(steps))
                        while si < target:
                            steps[si][0]()
                            si += 1
                t0 += ncp
            while si < len(steps):
                steps[si][0]()
                si += 1
    nc.finalize()
    return nc


def make_weights(coef_re, coef_im, factor):
    """wts[p, 128b + c] = g[c, 128b + 127 - p]; g = truncated gammatone IR."""
    cr = np.asarray(coef_re, np.float64)
    ci = np.asarray(coef_im, np.float64)
    f = np.asarray(factor, np.float64)
    c = cr + 1j * ci
    n = np.arange(NTAP)
    binom = (n + 3) * (n + 2) * (n + 1) / 6.0
    g = f[:, None] * np.real(binom[None, :] * c[:, None] ** n[None, :])
    g = g.astype(np.float32)
    wv = np.zeros((128, NTAP), np.float32)
    for b in range(NB):
        blk = g[:, 128 * b:128 * b + 128]
        wv[:, 128 * b:128 * b + 128] = blk[:, ::-1].T
    return wv.astype(ml_dtypes.bfloat16)


def make_tables(coef_re, coef_im, factor):
    """Piece-local derotation tables; sqrt(factor) split over mod+demod."""
    cr = np.asarray(coef_re, np.float64)
    ci = np.asarray(coef_im, np.float64)
    f = np.asarray(factor, np.float64)
    lam = np.hypot(cr, ci)
    beta = np.arctan2(ci, cr)
    sf = np.sqrt(f)
    j = np.arange(KP, dtype=np.float64)
    ph = j[None, :] * beta[:, None]
    mcl = (sf[:, None] * np.cos(ph)).astype(np.float32)
    msl = (-sf[:, None] * np.sin(ph)).astype(np.float32)
    lam_tile = np.broadcast_to(lam.astype(np.float32)[:, None], (C, KP)).copy()
    kb = KP * beta
    ck = np.cos(kb).astype(np.float32)[:, None]
    sk = np.sin(kb).astype(np.float32)[:, None]
    return mcl, msl, lam_tile, ck, sk


_CACHED_NC = None


def kernel(inp, coef_re, coef_im, factor):
    global _CACHED_NC
    inp = np.ascontiguousarray(np.asarray(inp, np.float32))
    assert inp.shape == (B, T)
    wv = make_weights(coef_re, coef_im, factor)
    mcl, msl, lam_tile, ck, sk = make_tables(coef_re, coef_im, factor)
    xq = np.zeros((B, PAD + T), np.float32)
    xq[:, PAD:] = inp
    xq16 = xq.astype(ml_dtypes.bfloat16)
    xr = np.ascontiguousarray(inp[:, T1 - W:])        # [B, KSEC] f32

    if _CACHED_NC is None:
        _CACHED_NC = build_bass()
    nc = _CACHED_NC

    in_maps = [
        {"xq": xq16[i:i + 1, :], "wts": wv, "xr32": xr[i:i + 1, :],
         "mcl": mcl, "msl": msl, "lamt": lam_tile, "ck": ck, "sk": sk}
        for i in range(B)
    ]
    res = run_bass_kernel_spmd(nc, in_maps, core_ids=list(range(B)))
    out = np.stack([np.asarray(res.results[i]["out"]).T for i in range(B)])
    return np.ascontiguousarray(out.astype(np.float32))
